# revision 1
# baseline (speedup 1.0000x reference)
"""MoE expert-FFN kernel for Trainium2, expert-parallel across 8 NeuronCores.

Problem: out[t] = silu(x[t] @ W1[e_t]^T) @ W2[e_t]^T with
  E=64 experts, D=512, H=1024, T=256 tokens.

Strategy (memory-bound on expert weights, ~268MB fp32 total):
  - Core c owns experts [8c, 8c+8). Host routes tokens to the core owning
    their expert (the hint's all-to-all done on host since we hold full
    inputs), padding each expert's tokens to a fixed capacity C.
  - Host pre-packs weights into the exact SBUF layout so the device does
    nothing but stream 4MiB/expert with perfect 128-partition DMAs.
  - On device, weights are the MOVING matmul operand (N=512 columns,
    full-rate float32r) and the tiny token blocks are the stationary
    operand, so the PE streams each weight element exactly once:
       H = silu(W1T-tiles streamed against x^T)     [tok, 1024] in PSUM
       H^T via 8 PE-transposes                      [128, tok] chunks
       Y = W2T-tiles streamed against H^T           [tok, 512]
  - float32r: full 4-byte weights in HBM (memory regime unchanged) with
    single-pass PE streaming; ~1.8e-4 absmax-relative vs the fp32 oracle.
"""

import numpy as np

E, D, H, T = 64, 512, 1024, 256
NCORES = 8
EPC = E // NCORES          # experts per core
DC = D // 128              # 4 d-chunks
HC = H // 128              # 8 h-chunks
WCOLS = DC * H + HC * D    # 8192 free columns of packed weights per expert
CB = 32                    # token block (PE-transpose granularity)

_prog_cache = {}


def _build_program(C, w_bufs=6, wdt_name="f32r"):
    import concourse.mybir as mybir
    import concourse.tile as tile
    from concourse import bacc

    f32 = mybir.dt.float32
    wdt = {"f32": f32, "f32r": mybir.dt.float32r,
           "bf16": mybir.dt.bfloat16, "f16": mybir.dt.float16}[wdt_name]
    blocks = C // CB
    nc = bacc.Bacc("TRN2", target_bir_lowering=False, debug=False)

    wts = nc.dram_tensor("wts", [EPC, 128, WCOLS], wdt, kind="ExternalInput")
    xt = nc.dram_tensor("xt", [128, EPC * DC * C], wdt, kind="ExternalInput")
    idt = nc.dram_tensor("idt", [CB, CB], wdt, kind="ExternalInput")
    yt = nc.dram_tensor("yt", [EPC, blocks, CB, D], f32, kind="ExternalOutput")

    with tile.TileContext(nc) as tc:
        with (
            tc.tile_pool(name="wpool", bufs=w_bufs) as wpool,
            tc.tile_pool(name="xpool", bufs=1) as xpool,
            tc.tile_pool(name="cpool", bufs=1) as cpool,
            tc.tile_pool(name="hpool", bufs=2) as hpool,
            tc.tile_pool(name="ypool", bufs=2) as ypool,
            tc.tile_pool(name="psh", bufs=2, space="PSUM") as pshp,
            tc.tile_pool(name="pst", bufs=2, space="PSUM") as pstp,
            tc.tile_pool(name="psy", bufs=2, space="PSUM") as psyp,
        ):
            ident = cpool.tile([CB, CB], wdt)
            nc.sync.dma_start(ident[:], idt[:])
            ident_w = ident[:]
            xall = xpool.tile([128, EPC * DC * C], wdt)
            nc.sync.dma_start(xall[:], xt[:])

            for s in range(EPC):
                w1 = wpool.tile([128, DC * H], wdt, tag="w")
                nc.sync.dma_start(w1[:], wts[s][:, :DC * H])
                w2 = wpool.tile([128, HC * D], wdt, tag="w")
                nc.sync.dma_start(w2[:], wts[s][:, DC * H:])

                for b in range(blocks):
                    # ---- fc1: Hpre[t, h] = sum_d x^T[d, t] * W1T[d, h]
                    psh = pshp.tile([CB, H], f32, tag="psh")
                    for nh in range(2):
                        for c in range(DC):
                            nc.tensor.matmul(
                                psh[:, nh * 512:(nh + 1) * 512],
                                xall[:, (s * DC + c) * C + b * CB:
                                     (s * DC + c) * C + (b + 1) * CB],
                                w1[:, c * H + nh * 512: c * H + (nh + 1) * 512],
                                start=(c == 0),
                                stop=(c == DC - 1),
                            )

                    # ---- silu: h = psh * sigmoid(psh)   [CB, 1024] -> SBUF
                    sig = hpool.tile([CB, H], f32, tag="sig")
                    nc.scalar.activation(
                        sig[:], psh[:], mybir.ActivationFunctionType.Sigmoid
                    )
                    hbuf = hpool.tile([CB, H], wdt, tag="h")
                    nc.vector.tensor_mul(hbuf[:], psh[:], sig[:])

                    # ---- transpose h -> hT [128, HC*CB] via PE
                    pst = pstp.tile([128, HC * CB], wdt, tag="pst")
                    for ch in range(HC):
                        nc.tensor.transpose(
                            pst[:, ch * CB:(ch + 1) * CB],
                            hbuf[:, ch * 128:(ch + 1) * 128],
                            ident_w,
                        )
                    ht = hpool.tile([128, HC * CB], wdt, tag="ht")
                    nc.vector.tensor_copy(ht[:], pst[:])

                    # ---- fc2: Y[t, d] = sum_h hT[h, t] * W2T[h, d]
                    psy = psyp.tile([CB, D], f32, tag="psy")
                    for ch in range(HC):
                        nc.tensor.matmul(
                            psy[:],
                            ht[:, ch * CB:(ch + 1) * CB],
                            w2[:, ch * D: (ch + 1) * D],
                            start=(ch == 0),
                            stop=(ch == HC - 1),
                        )

                    ybuf = ypool.tile([CB, D], f32, tag="y")
                    nc.vector.tensor_copy(ybuf[:], psy[:])
                    nc.scalar.dma_start(yt[s, b], ybuf[:])

    nc.compile()
    return nc


def _route(expert_idx):
    idx = np.asarray(expert_idx).astype(np.int64)
    order = np.argsort(idx, kind="stable")
    counts = np.bincount(idx, minlength=E)
    starts = np.zeros(E + 1, dtype=np.int64)
    starts[1:] = np.cumsum(counts)
    return order, starts, counts


def _pack_inputs(x, fc1_w, fc2_w, order, starts, C, np_dtype=np.float32):
    in_maps = []
    for core in range(NCORES):
        wh = np.empty((EPC, 128, WCOLS), np_dtype)
        xh = np.zeros((128, EPC * DC * C), np_dtype)
        for s in range(EPC):
            e = core * EPC + s
            # W1^T = fc1_w[e].T : [D, H]; d = c*128 + p -> col c*H + h
            w1t = np.ascontiguousarray(fc1_w[e].T).reshape(DC, 128, H)
            wh[s, :, :DC * H] = w1t.transpose(1, 0, 2).reshape(128, DC * H)
            # W2^T = fc2_w[e].T : [H, D]; h = ch*128 + p -> col DC*H + ch*D + d
            w2t = np.ascontiguousarray(fc2_w[e].T).reshape(HC, 128, D)
            wh[s, :, DC * H:] = w2t.transpose(1, 0, 2).reshape(128, HC * D)

            toks = order[starts[e]:starts[e + 1]]
            n = len(toks)
            if n:
                xte = np.ascontiguousarray(x[toks].T).reshape(DC, 128, n)
                for c in range(DC):
                    base = (s * DC + c) * C
                    xh[:, base:base + n] = xte[c]
        in_maps.append({"wts": wh, "xt": xh,
                        "idt": np.eye(CB, dtype=np_dtype)})
    return in_maps


def _unpack_outputs(results, order, starts, C, out_dtype):
    out = np.zeros((T, D), out_dtype)
    for core in range(NCORES):
        yh = np.asarray(results[core]["yt"]).reshape(EPC, C, D)
        for s in range(EPC):
            e = core * EPC + s
            toks = order[starts[e]:starts[e + 1]]
            n = len(toks)
            if n:
                out[toks] = yh[s, :n]
    return out


def kernel(x, expert_idx, fc1_w, fc2_w):
    from concourse.bass_utils import run_bass_kernel_spmd

    x = np.asarray(x, dtype=np.float32)
    fc1_w = np.asarray(fc1_w, dtype=np.float32)
    fc2_w = np.asarray(fc2_w, dtype=np.float32)

    order, starts, counts = _route(expert_idx)
    C = max(CB, int(-(-int(counts.max()) // CB) * CB))

    if C not in _prog_cache:
        _prog_cache[C] = _build_program(C)
    nc = _prog_cache[C]

    in_maps = _pack_inputs(x, fc1_w, fc2_w, order, starts, C)
    res = run_bass_kernel_spmd(nc, in_maps, list(range(NCORES)))
    return _unpack_outputs(res.results, order, starts, C, np.float32)



# revision 2
# speedup vs baseline: 1.7289x; 1.7289x over previous
"""MoE expert-FFN kernel for Trainium2, expert-parallel across 8 NeuronCores.

Problem: out[t] = silu(x[t] @ W1[e_t]^T) @ W2[e_t]^T with
  E=64 experts, D=512, H=1024, T=256 tokens.

Strategy (memory-bound on expert weights):
  - Core c owns experts [8c, 8c+8). Host routes tokens to the core owning
    their expert, padding each expert's tokens to a fixed capacity C.
  - Host pre-packs weights into the exact SBUF layout in FP16 (halves the
    HBM traffic vs fp32; PSUM accumulation stays fp32, absmax rel err
    ~1e-3 vs the fp32 oracle, well inside the 2e-2 budget).
  - Experts are processed in groups of 4 so that every on-chip tile is a
    full 128-partition tile (4 experts x 32-token capacity):
      fc1: per expert, weights are the MOVING matmul operand streamed
           against the stationary token block; the 4 experts of a group
           stream concurrently through distinct 32-column PE groups
           (tile_position=(0,32s)).
      silu: one ACT op per group, PSUM [128,1024] -> SBUF fp16.
      transpose: one [128,128] PE transpose per h-chunk (8 per group).
      fc2: same col-tiled layout, accumulating over 8 h-chunks.
  - Weights arrive as 4 x 4MiB DMAs (one per group x {W1,W2}) on the sync
    HWDGE ring; output leaves on the scalar ring.
"""

import numpy as np

E, D, H, T = 64, 512, 1024, 256
NCORES = 8
EPC = E // NCORES          # experts per core (8)
GPE = 4                    # experts per group
NG = EPC // GPE            # groups per core (2)
DC = D // 128              # 4 d-chunks
HC = H // 128              # 8 h-chunks
WE = DC * H                # 4096 cols of packed W1 (= HC*D for W2) per expert
CB = 32                    # token block (col-tile granularity)

_prog_cache = {}


def _build_program(C):
    import concourse.mybir as mybir
    import concourse.tile as tile
    from concourse import bacc

    f32 = mybir.dt.float32
    f16 = mybir.dt.float16
    blocks = C // CB
    nc = bacc.Bacc("TRN2", target_bir_lowering=False, debug=False)

    # [group, {w1,w2}, 128, 4 experts * 4096 cols]
    wts = nc.dram_tensor("wts", [NG, 2, 128, GPE * WE], f16, kind="ExternalInput")
    xt = nc.dram_tensor("xt", [128, EPC * DC * C], f16, kind="ExternalInput")
    idt = nc.dram_tensor("idt", [128, 128], f16, kind="ExternalInput")
    yt = nc.dram_tensor("yt", [NG, blocks, 128, D], f16, kind="ExternalOutput")

    with tile.TileContext(nc) as tc:
        with (
            tc.tile_pool(name="wpool", bufs=4) as wpool,
            tc.tile_pool(name="xpool", bufs=1) as xpool,
            tc.tile_pool(name="cpool", bufs=1) as cpool,
            tc.tile_pool(name="hpool", bufs=2) as hpool,
            tc.tile_pool(name="tpool", bufs=2) as tpool,
            tc.tile_pool(name="ypool", bufs=2) as ypool,
            tc.tile_pool(name="psh", bufs=2, space="PSUM") as pshp,
            tc.tile_pool(name="pst", bufs=2, space="PSUM") as pstp,
            tc.tile_pool(name="psy", bufs=2, space="PSUM") as psyp,
        ):
            ident = cpool.tile([128, 128], f16)
            nc.sync.dma_start(ident[:], idt[:])
            ident_w = ident[:]
            xall = xpool.tile([128, EPC * DC * C], f16)
            nc.sync.dma_start(xall[:], xt[:])

            for g in range(NG):
                w1 = wpool.tile([128, GPE * WE], f16, tag="w")
                nc.sync.dma_start(w1[:], wts[g, 0])
                w2 = wpool.tile([128, GPE * WE], f16, tag="w")
                nc.sync.dma_start(w2[:], wts[g, 1])

                for b in range(blocks):
                    # ---- fc1: psh[32s+t, h] = sum_d x^T[d, t] * W1T[d, h]
                    # 4 experts stream concurrently through distinct
                    # 32-column PE groups.
                    psh = pshp.tile([128, H], f32, tag="psh")
                    for c in range(DC):
                        for s in range(GPE):
                            sg = g * GPE + s
                            xsl = xall[:, (sg * DC + c) * C + b * CB:
                                       (sg * DC + c) * C + (b + 1) * CB]
                            for nh in range(2):
                                nc.tensor.matmul(
                                    psh[32 * s:32 * (s + 1),
                                        nh * 512:(nh + 1) * 512],
                                    xsl,
                                    w1[:, s * WE + c * H + nh * 512:
                                       s * WE + c * H + (nh + 1) * 512],
                                    start=(c == 0),
                                    stop=(c == DC - 1),
                                    tile_position=(0, 32 * s),
                                )

                    # ---- silu: [128, 1024] PSUM f32 -> SBUF fp16
                    hbuf = hpool.tile([128, H], f16, tag="h")
                    nc.scalar.activation(
                        hbuf[:], psh[:], mybir.ActivationFunctionType.Silu
                    )

                    # ---- transpose: [128 tok, 128 h] -> [128 h, 128 tok]
                    pst = pstp.tile([128, H], f16, tag="pst")
                    for ch in range(HC):
                        nc.tensor.transpose(
                            pst[:, ch * 128:(ch + 1) * 128],
                            hbuf[:, ch * 128:(ch + 1) * 128],
                            ident_w,
                        )
                    ht = tpool.tile([128, H], f16, tag="ht")
                    nc.vector.tensor_copy(ht[:], pst[:])

                    # ---- fc2: psy[32s+t, d] = sum_h hT[h, t] * W2T[h, d]
                    psy = psyp.tile([128, D], f32, tag="psy")
                    for ch in range(HC):
                        for s in range(GPE):
                            nc.tensor.matmul(
                                psy[32 * s:32 * (s + 1), :],
                                ht[:, ch * 128 + 32 * s:ch * 128 + 32 * (s + 1)],
                                w2[:, s * WE + ch * D:s * WE + (ch + 1) * D],
                                start=(ch == 0),
                                stop=(ch == HC - 1),
                                tile_position=(0, 32 * s),
                            )

                    ybuf = ypool.tile([128, D], f16, tag="y")
                    nc.vector.tensor_copy(ybuf[:], psy[:])
                    nc.scalar.dma_start(yt[g, b], ybuf[:])

    nc.compile()
    return nc


def _route(expert_idx):
    idx = np.asarray(expert_idx).astype(np.int64)
    order = np.argsort(idx, kind="stable")
    counts = np.bincount(idx, minlength=E)
    starts = np.zeros(E + 1, dtype=np.int64)
    starts[1:] = np.cumsum(counts)
    return order, starts, counts


def _pack_inputs(x, fc1_w, fc2_w, order, starts, C):
    x16 = x.astype(np.float16)
    in_maps = []
    for core in range(NCORES):
        wh = np.empty((NG, 2, 128, GPE * WE), np.float16)
        xh = np.zeros((128, EPC * DC * C), np.float16)
        for s in range(EPC):
            e = core * EPC + s
            g, sl = divmod(s, GPE)
            # W1^T = fc1_w[e].T : [D, H]; d = c*128 + p -> col c*H + h
            w1t = np.ascontiguousarray(fc1_w[e].T).reshape(DC, 128, H)
            wh[g, 0, :, sl * WE:(sl + 1) * WE] = (
                w1t.transpose(1, 0, 2).reshape(128, WE).astype(np.float16))
            # W2^T = fc2_w[e].T : [H, D]; h = ch*128 + p -> col ch*D + d
            w2t = np.ascontiguousarray(fc2_w[e].T).reshape(HC, 128, D)
            wh[g, 1, :, sl * WE:(sl + 1) * WE] = (
                w2t.transpose(1, 0, 2).reshape(128, WE).astype(np.float16))

            toks = order[starts[e]:starts[e + 1]]
            n = len(toks)
            if n:
                xte = np.ascontiguousarray(x16[toks].T).reshape(DC, 128, n)
                for c in range(DC):
                    base = (s * DC + c) * C
                    xh[:, base:base + n] = xte[c]
        in_maps.append({"wts": wh, "xt": xh,
                        "idt": np.eye(128, dtype=np.float16)})
    return in_maps


def _unpack_outputs(results, order, starts, C, out_dtype):
    blocks = C // CB
    out = np.zeros((T, D), out_dtype)
    for core in range(NCORES):
        # [NG, blocks, 128, D] -> per expert slot rows 32*sl..32*(sl+1)
        yh = np.asarray(results[core]["yt"]).astype(out_dtype)
        for s in range(EPC):
            e = core * EPC + s
            g, sl = divmod(s, GPE)
            toks = order[starts[e]:starts[e + 1]]
            n = len(toks)
            if n:
                ye = yh[g, :, 32 * sl:32 * (sl + 1), :].reshape(C, D)
                out[toks] = ye[:n]
    return out


def kernel(x, expert_idx, fc1_w, fc2_w):
    from concourse.bass_utils import run_bass_kernel_spmd

    x = np.asarray(x, dtype=np.float32)
    fc1_w = np.asarray(fc1_w, dtype=np.float32)
    fc2_w = np.asarray(fc2_w, dtype=np.float32)

    order, starts, counts = _route(expert_idx)
    C = max(CB, int(-(-int(counts.max()) // CB) * CB))

    if C not in _prog_cache:
        _prog_cache[C] = _build_program(C)
    nc = _prog_cache[C]

    in_maps = _pack_inputs(x, fc1_w, fc2_w, order, starts, C)
    res = run_bass_kernel_spmd(nc, in_maps, list(range(NCORES)))
    return _unpack_outputs(res.results, order, starts, C, np.float32)
